# revision 5
# baseline (speedup 1.0000x reference)
# GCN layer kernel for Trainium2: out[b] = relu((a[b] @ x[b]) @ W) * mask[b]
#
# Sharding: data-parallel over the batch (graph) dim. B=8 graphs, 8 cores,
# one graph per core; W replicated. Inputs are the FULL tensors; shards are
# sliced host-side and the per-core outputs stacked back together.
#
# Per-core dataflow (a: [2048,2048], x: [2048,512], W: [512,512]):
#   - All matmuls run in bf16 (1 cycle/row on the PE, same rate as f32r, and
#     rel-err ~2e-3 against the fp32 reference -- the tolerance is 2e-2).
#   - a is cast fp32->bf16 IN FLIGHT by gpsimd (software-DGE) DMAs, then
#     transposed SBUF->SBUF by the DMA xbar (dma_start(transpose=True), a
#     16x128-tile crossbar). One xbar instruction per [128,2048] strip
#     yields the tiled transpose at[:, mtile, ni, :]; the PE does NO
#     transpose work at all, only the 320 real matmuls:
#       mm1: tT[f,n] = sum_m x[m,f] * aT[m,n]   (lhsT = x, rhs = aT)
#       mm2: out[n,d] = sum_f tT[f,n] * W[f,d]  (lhsT = tT, rhs = W)
#     = (64 + 16) matmuls x 512 cols x 4 chunks = 164k PE cycles ~= 68us.
#   - x, W load as fp32 on the sync queue and are cast to bf16 on DVE.
#   - mask[n] = any(x[n,:] != 0) via |x| row-sums (ACT, from bf16 x),
#     applied as the scale input of the fused ReLU.
#
# Schedule (chunk = 512 output rows, 4 chunks, software-pipelined):
#   gpsimd queue: ab chunk loads (cast-DMA), then output stores
#   scalar queue: xbar transposes ab -> at (4 per chunk, prefetched)
#   sync queue:   x, w fp32 loads
#   PE: per chunk, 64 mm1 + 16 mm2-of-previous-chunk, no gaps; mm2 runs a
#     full chunk behind mm1 so its tt inputs (PSUM->SBUF bf16 copybacks,
#     alternating DVE/ACT) are always long since landed.
#   PSUM: 4 mm1 banks + 4 mm2 banks. Warm-up matmuls (identity fp32 +
#     bf16 warms hanging off the x-piece casts) keep the HAM clock-gate
#     open through the initial DMA wait; they borrow mm2's idle banks.

import numpy as np

B, N, F, D = 8, 2048, 512, 512
P = 128
NT = N // P        # 16 row-tiles of n (and of m, since a is square)
FT = F // P        # 4 tiles of f
NCHUNK = 512       # n is processed in chunks of 512 rows
NJ = N // NCHUNK   # 4
NSUB = NCHUNK // P # 4

_CACHE = {}


def _build_nc():
    from contextlib import ExitStack

    from concourse import bacc, mybir, tile
    from concourse.masks import make_identity

    f32 = mybir.dt.float32
    bf16 = mybir.dt.bfloat16
    AF = mybir.ActivationFunctionType

    nc = bacc.Bacc(None)
    a_d = nc.dram_tensor("a", [N, N], f32, kind="ExternalInput")
    x_d = nc.dram_tensor("x", [N, F], f32, kind="ExternalInput")
    w_d = nc.dram_tensor("kernel", [F, D], f32, kind="ExternalInput")
    o_d = nc.dram_tensor("out", [N, D], f32, kind="ExternalOutput")

    with tile.TileContext(nc) as tc, ExitStack() as ctx:
        const = ctx.enter_context(tc.tile_pool(name="const", bufs=1))
        xp = ctx.enter_context(tc.tile_pool(name="xp", bufs=1))
        wp = ctx.enter_context(tc.tile_pool(name="wp", bufs=1))
        xs = ctx.enter_context(tc.tile_pool(name="xs", bufs=3))
        ab_pool = ctx.enter_context(tc.tile_pool(name="ab_pool", bufs=9))
        atp = ctx.enter_context(tc.tile_pool(name="atp", bufs=2))
        ttp = ctx.enter_context(tc.tile_pool(name="ttp", bufs=2))
        outp = ctx.enter_context(tc.tile_pool(name="outp", bufs=4))
        scr = ctx.enter_context(tc.tile_pool(name="scr", bufs=2))
        ps_mm = ctx.enter_context(tc.tile_pool(name="ps_mm", bufs=4, space="PSUM"))
        ps_o = ctx.enter_context(tc.tile_pool(name="ps_o", bufs=4, space="PSUM"))

        ident = const.tile([P, P], f32)
        make_identity(nc, ident[:])

        def warm_fp32():
            # fp32 identity matmul: registers as HAM activity, output unused.
            # Borrows an mm2 PSUM slot (idle until ~35us in).
            pw = ps_o.tile([P, D], f32, tag="pso", name="pw")
            nc.tensor.matmul(
                pw[:, :P], lhsT=ident[:], rhs=ident[:], start=True, stop=True
            )

        def warm_bf16(op):
            # bf16 warm matmul whose operand is a freshly-cast x piece: fires
            # as the piece lands, trickling PE activity through the DMA wait.
            pw = ps_o.tile([P, D], f32, tag="pso", name="pwb")
            nc.tensor.matmul(pw[:, :P], lhsT=op, rhs=op, start=True, stop=True)

        for _ in range(12):
            warm_fp32()

        # x: fp32 pieces on the sync queue -> DVE cast into the resident bf16
        # tile [p, o, f] (m = o*128+p on partitions). ACT accumulates the
        # per-row |x| sums for the padded-row mask as each piece lands.
        x_b = xp.tile([P, NT, F], bf16)
        sumabs = const.tile([P, NT], f32)
        mask_sb = const.tile([P, NT], f32)

        for piece in range(8):
            xl = xs.tile([P, 2, F], f32, tag="xl", name="xl")
            nc.sync.dma_start(
                xl[:],
                x_d[piece * 2 * P : (piece + 1) * 2 * P, :].rearrange(
                    "(o p) f -> p o f", p=P
                ),
            )
            nc.vector.tensor_copy(x_b[:, piece * 2 : piece * 2 + 2, :], xl[:])
            warm_bf16(x_b[:, piece * 2, 0:P])
            for o in (piece * 2, piece * 2 + 1):
                abs_scr = scr.tile([P, F], bf16, tag="abs_scr")
                nc.scalar.activation(
                    abs_scr[:], x_b[:, o, :], AF.Abs,
                    accum_out=sumabs[:, o : o + 1],
                )

        # w: fp32 on sync queue -> DVE cast to bf16 [p, ftile, d]
        w_b = wp.tile([P, FT, D], bf16)
        wl = xs.tile([P, FT, D], f32, tag="xl", name="wl")
        nc.sync.dma_start(wl[:], w_d[:].rearrange("(o p) d -> p o d", p=P))
        nc.vector.tensor_copy(w_b[:], wl[:])

        nc.vector.tensor_scalar(
            mask_sb[:], sumabs[:], 0.0, None, mybir.AluOpType.is_gt
        )

        # a: per chunk, 4 strips [128, 2048] cast-loaded to bf16 by gpsimd
        # DMAs, then one xbar transpose per strip into at[p=m%128, mtile,
        # ni, r=n%128] on the scalar queue.
        def load_ab(nj):
            strips = []
            for ni in range(NSUB):
                ab = ab_pool.tile([P, N], bf16, tag="ab", name="ab")
                r0 = (nj * NSUB + ni) * P
                nc.gpsimd.dma_start(ab[:], a_d[r0 : r0 + P, :])
                strips.append(ab)
            return strips

        def transpose_ab(strips):
            at = atp.tile([P, NT, NSUB, P], bf16, tag="at", name="at")
            for ni in range(NSUB):
                nc.scalar.dma_start(at[:, :, ni, :], strips[ni][:], transpose=True)
            return at

        ab_chunks = [load_ab(0), load_ab(1)]
        at_cur = transpose_ab(ab_chunks[0])

        cb = 0  # copyback counter for DVE/ACT alternation

        def copyback(dst, src):
            nonlocal cb
            if cb % 2 == 0:
                nc.vector.tensor_copy(dst, src)
            else:
                nc.scalar.copy(dst, src)
            cb += 1

        tts = [None] * NJ
        po_banks = {}

        def mm2_group(nj, fi):
            # accumulate po[ns] += ttT[fi] @ W[fi]; on the last fi, fused
            # relu(mask * po) -> SBUF -> store via the gpsimd queue.
            tt = tts[nj]
            if fi == 0:
                po_banks[nj] = [
                    ps_o.tile([P, D], f32, tag="pso", name=f"po_{nj}_{ns}")
                    for ns in range(NSUB)
                ]
            for ns in range(NSUB):
                nc.tensor.matmul(
                    po_banks[nj][ns][:],
                    lhsT=tt[:, fi, ns * P : (ns + 1) * P],
                    rhs=w_b[:, fi],
                    start=(fi == 0),
                    stop=(fi == FT - 1),
                )
            if fi == FT - 1:
                for ns in range(NSUB):
                    po = po_banks[nj][ns]
                    ni = nj * NSUB + ns
                    ob = outp.tile([P, D], f32, tag="ob")
                    nc.scalar.activation(
                        ob[:], po[:], AF.Relu, scale=mask_sb[:, ni : ni + 1]
                    )
                    nc.gpsimd.dma_start(o_d[ni * P : (ni + 1) * P, :], ob[:])

        for nj in range(NJ):
            at_next = None
            if nj + 1 < NJ:
                at_next = transpose_ab(ab_chunks[nj + 1])
            if nj + 2 < NJ:
                ab_chunks.append(load_ab(nj + 2))

            tt = ttp.tile([P, FT, NCHUNK], bf16, tag="tt")
            tts[nj] = tt
            pt = [
                ps_mm.tile([P, NCHUNK], f32, tag="psm", name=f"pt_{nj}_{fi}")
                for fi in range(FT)
            ]
            for fi in range(FT):
                for mi in range(NT):
                    nc.tensor.matmul(
                        pt[fi][:],
                        lhsT=x_b[:, mi, fi * P : (fi + 1) * P],
                        rhs=at_cur[:, mi, :, :],
                        start=(mi == 0),
                        stop=(mi == NT - 1),
                    )
                copyback(tt[:, fi], pt[fi][:])
                # mm2 of the previous chunk, one fi-group per mm1 fi-block:
                # inputs landed a full chunk ago, so the PE never waits.
                if nj > 0:
                    mm2_group(nj - 1, fi)

            at_cur = at_next

        for fi in range(FT):
            mm2_group(NJ - 1, fi)

    nc.compile()
    return nc


def get_nc():
    if "nc" not in _CACHE:
        _CACHE["nc"] = _build_nc()
    return _CACHE["nc"]


def kernel(**inputs) -> np.ndarray:
    from concourse.bass_utils import run_bass_kernel_spmd

    x = np.ascontiguousarray(np.asarray(inputs["x"], dtype=np.float32))
    a = np.ascontiguousarray(np.asarray(inputs["a"], dtype=np.float32))
    w = np.ascontiguousarray(np.asarray(inputs["kernel"], dtype=np.float32))
    assert x.shape == (B, N, F) and a.shape == (B, N, N) and w.shape == (F, D)

    nc = get_nc()
    in_maps = [{"a": a[b], "x": x[b], "kernel": w} for b in range(B)]
    res = run_bass_kernel_spmd(nc, in_maps, core_ids=list(range(B)))
    return np.stack([res.results[b]["out"] for b in range(B)], axis=0)


# revision 7
# speedup vs baseline: 1.1377x; 1.1377x over previous
# GCN layer kernel for Trainium2: out[b] = relu((a[b] @ x[b]) @ W) * mask[b]
#
# Sharding: data-parallel over the batch (graph) dim. B=8 graphs, 8 cores,
# one graph per core; W replicated. Inputs are the FULL tensors; shards are
# sliced host-side and the per-core outputs stacked back together.
#
# Per-core dataflow (a: [2048,2048], x: [2048,512], W: [512,512]):
#   - All matmuls run in bf16 (1 cycle/row on the PE, same rate as f32r;
#     rel-err ~3e-3 against the fp32 reference -- tolerance is 2e-2).
#   - a loads as fp32 [128,2048] strips on the two HWDGE queues (sync +
#     scalar), is cast to bf16 on DVE, then transposed SBUF->SBUF by the
#     DMA xbar (dma_start(transpose=True)): one instruction per strip
#     yields the tiled transpose at[p=m%128, mtile, ni, r=n%128]. The PE
#     does NO transpose work, only the 320 real matmuls:
#       mm1: tT[f,n] = sum_m x[m,f] * aT[m,n]   (lhsT = x, rhs = aT)
#       mm2: out[n,d] = sum_f tT[f,n] * W[f,d]  (lhsT = tT, rhs = W)
#     = (64 + 16) matmuls x 512 cols x 4 chunks ~= 164k PE cycles ~= 68us.
#   - xbar transposes occupy their issuing engine ~1.6us each (measured),
#     so they are split 2+2 per chunk between the sync and scalar queues.
#   - x loads as 4 fp32 column-blocks (f-tiles) so mm1's fi=0 block can
#     start after one quarter of x; DVE casts into the resident bf16 tile.
#   - mask[n] = any(x[n,:] != 0) via |x| row-sums (ACT, from bf16 x),
#     applied as the scale input of the fused ReLU.
#
# Schedule (chunk = 512 output rows = 4 a-strips, software-pipelined):
#   sync queue:   a strips (c0: 2 strips; c1..c3: 4 each) + 2 xbar T/chunk
#   scalar queue: c0's other 2 strips, x blocks, w, + 2 xbar T/chunk
#   gpsimd queue: output stores only
#   DVE: a-strip casts, x/w casts, half the tt copybacks
#   ACT: |x| mask reductions, fused ReLU, half the tt copybacks
#   PE: per chunk, 64 mm1 + 16 mm2-of-previous-chunk; mm2 runs a full
#     chunk behind mm1 so its tt inputs are always long since landed.
#   PSUM: 4 mm1 banks + 4 mm2 banks. Warm-up matmuls (identity fp32 up
#     front, bf16 warms hanging off each cast) keep the HAM clock-gate
#     open through the initial DMA wait; they borrow mm2's idle banks.

import numpy as np

B, N, F, D = 8, 2048, 512, 512
P = 128
NT = N // P        # 16 row-tiles of n (and of m, since a is square)
FT = F // P        # 4 tiles of f
NCHUNK = 512       # n is processed in chunks of 512 rows
NJ = N // NCHUNK   # 4
NSUB = NCHUNK // P # 4

_CACHE = {}


def _build_nc():
    from contextlib import ExitStack

    from concourse import bacc, mybir, tile
    from concourse.masks import make_identity

    f32 = mybir.dt.float32
    bf16 = mybir.dt.bfloat16
    AF = mybir.ActivationFunctionType

    nc = bacc.Bacc(None)
    a_d = nc.dram_tensor("a", [N, N], f32, kind="ExternalInput")
    x_d = nc.dram_tensor("x", [N, F], f32, kind="ExternalInput")
    w_d = nc.dram_tensor("kernel", [F, D], f32, kind="ExternalInput")
    o_d = nc.dram_tensor("out", [N, D], f32, kind="ExternalOutput")

    with tile.TileContext(nc) as tc, ExitStack() as ctx:
        const = ctx.enter_context(tc.tile_pool(name="const", bufs=1))
        xp = ctx.enter_context(tc.tile_pool(name="xp", bufs=1))
        wp = ctx.enter_context(tc.tile_pool(name="wp", bufs=1))
        xs = ctx.enter_context(tc.tile_pool(name="xs", bufs=2))
        ws = ctx.enter_context(tc.tile_pool(name="ws", bufs=1))
        afp = ctx.enter_context(tc.tile_pool(name="afp", bufs=4))
        abp = ctx.enter_context(tc.tile_pool(name="abp", bufs=9))
        atp = ctx.enter_context(tc.tile_pool(name="atp", bufs=3))
        ttp = ctx.enter_context(tc.tile_pool(name="ttp", bufs=2))
        outp = ctx.enter_context(tc.tile_pool(name="outp", bufs=4))
        scr = ctx.enter_context(tc.tile_pool(name="scr", bufs=2))
        ps_mm = ctx.enter_context(tc.tile_pool(name="ps_mm", bufs=4, space="PSUM"))
        ps_o = ctx.enter_context(tc.tile_pool(name="ps_o", bufs=4, space="PSUM"))

        ident = const.tile([P, P], f32)
        make_identity(nc, ident[:])

        def warm_fp32():
            # fp32 identity matmul: registers as HAM activity, output unused.
            # Borrows an mm2 PSUM slot (idle until ~30us in).
            pw = ps_o.tile([P, D], f32, tag="pso", name="pw")
            nc.tensor.matmul(
                pw[:, :P], lhsT=ident[:], rhs=ident[:], start=True, stop=True
            )

        def warm_bf16(op):
            # bf16 warm matmul on a freshly-cast tile: fires as the cast
            # lands, trickling PE activity through the DMA wait.
            pw = ps_o.tile([P, D], f32, tag="pso", name="pwb")
            nc.tensor.matmul(pw[:, :P], lhsT=op, rhs=op, start=True, stop=True)

        for _ in range(12):
            warm_fp32()

        x_b = xp.tile([P, NT, F], bf16)
        w_b = wp.tile([P, FT, D], bf16)
        sumabs = const.tile([P, NT], f32)
        mask_sb = const.tile([P, NT], f32)

        # ---- a-strip load (fp32) / cast (DVE) / xbar transpose helpers ----
        ab_strips = [[None] * NSUB for _ in range(NJ)]
        at_tiles = [None] * NJ

        def load_strip(nj, ni, queue):
            af = afp.tile([P, N], f32, tag="af", name="af")
            r0 = (nj * NSUB + ni) * P
            queue.dma_start(af[:], a_d[r0 : r0 + P, :])
            return af

        def cast_strip(nj, ni, af, warm=False):
            ab = abp.tile([P, N], bf16, tag="ab", name="ab")
            nc.vector.tensor_copy(ab[:], af[:])
            ab_strips[nj][ni] = ab
            if warm:
                warm_bf16(ab[:, 0:P])

        def xbar_T(nj, ni, queue):
            # in [128(n), 2048(m)] bf16 strip -> out[p=m%128, mtile, r=n%128]
            if at_tiles[nj] is None:
                at_tiles[nj] = atp.tile([P, NT, NSUB, P], bf16, tag="at", name="at")
            queue.dma_start(
                at_tiles[nj][:, :, ni, :], ab_strips[nj][ni][:], transpose=True
            )

        def load_xq(fi):
            xl = xs.tile([P, NT, P], f32, tag="xl", name="xl")
            nc.scalar.dma_start(
                xl[:],
                x_d[:, fi * P : (fi + 1) * P].rearrange("(o p) f -> p o f", p=P),
            )
            return xl

        def cast_xq(fi, xl):
            nc.vector.tensor_copy(x_b[:, :, fi * P : (fi + 1) * P], xl[:])
            warm_bf16(x_b[:, 0, fi * P : (fi + 1) * P])

        # ---- preamble: chunk 0 split across both queues, x blocks, w ----
        af00 = load_strip(0, 0, nc.sync)
        af01 = load_strip(0, 1, nc.sync)
        af02 = load_strip(0, 2, nc.scalar)
        af03 = load_strip(0, 3, nc.scalar)
        xl0 = load_xq(0)
        cast_strip(0, 0, af00, warm=True)
        cast_strip(0, 1, af01)
        cast_strip(0, 2, af02, warm=True)
        cast_strip(0, 3, af03)
        xbar_T(0, 0, nc.sync)
        xbar_T(0, 1, nc.sync)
        cast_xq(0, xl0)
        xl1 = load_xq(1)
        xbar_T(0, 2, nc.scalar)
        xbar_T(0, 3, nc.scalar)
        cast_xq(1, xl1)
        # chunk 1 loads on sync; x blocks 2,3 + w on scalar
        af1 = [load_strip(1, ni, nc.sync) for ni in range(NSUB)]
        xl2 = load_xq(2)
        xl3 = load_xq(3)
        wl = ws.tile([P, FT, D], f32, tag="wl", name="wl")
        nc.scalar.dma_start(wl[:], w_d[:].rearrange("(o p) d -> p o d", p=P))
        cast_strip(1, 0, af1[0], warm=True)
        cast_strip(1, 1, af1[1])
        cast_xq(2, xl2)
        cast_strip(1, 2, af1[2])
        cast_strip(1, 3, af1[3])
        cast_xq(3, xl3)
        nc.vector.tensor_copy(w_b[:], wl[:])
        xbar_T(1, 0, nc.sync)
        xbar_T(1, 1, nc.sync)
        xbar_T(1, 2, nc.scalar)
        xbar_T(1, 3, nc.scalar)

        # mask reductions after x_b is assembled (ACT is otherwise idle
        # until the first ReLU at ~30us)
        for o in range(NT):
            abs_scr = scr.tile([P, F], bf16, tag="abs_scr")
            nc.scalar.activation(
                abs_scr[:], x_b[:, o, :], AF.Abs, accum_out=sumabs[:, o : o + 1]
            )
        nc.vector.tensor_scalar(
            mask_sb[:], sumabs[:], 0.0, None, mybir.AluOpType.is_gt
        )

        # ---- main loop ----
        cb = 0  # copyback counter for DVE/ACT alternation

        def copyback(dst, src):
            nonlocal cb
            if cb % 2 == 0:
                nc.vector.tensor_copy(dst, src)
            else:
                nc.scalar.copy(dst, src)
            cb += 1

        tts = [None] * NJ
        po_banks = {}

        def mm2_group(nj, fi):
            # accumulate po[ns] += ttT[fi] @ W[fi]; on the last fi, fused
            # relu(mask * po) -> SBUF -> store via the gpsimd queue.
            tt = tts[nj]
            if fi == 0:
                po_banks[nj] = [
                    ps_o.tile([P, D], f32, tag="pso", name=f"po_{nj}_{ns}")
                    for ns in range(NSUB)
                ]
            for ns in range(NSUB):
                nc.tensor.matmul(
                    po_banks[nj][ns][:],
                    lhsT=tt[:, fi, ns * P : (ns + 1) * P],
                    rhs=w_b[:, fi],
                    start=(fi == 0),
                    stop=(fi == FT - 1),
                )
            if fi == FT - 1:
                for ns in range(NSUB):
                    po = po_banks[nj][ns]
                    ni = nj * NSUB + ns
                    ob = outp.tile([P, D], f32, tag="ob")
                    nc.scalar.activation(
                        ob[:], po[:], AF.Relu, scale=mask_sb[:, ni : ni + 1]
                    )
                    nc.gpsimd.dma_start(o_d[ni * P : (ni + 1) * P, :], ob[:])

        for nj in range(NJ):
            # prefetch chunk nj+2: loads on sync, casts on DVE, T split 2+2
            if nj + 2 < NJ:
                af_n = [load_strip(nj + 2, ni, nc.sync) for ni in range(NSUB)]
                for ni in range(NSUB):
                    cast_strip(nj + 2, ni, af_n[ni])
                xbar_T(nj + 2, 0, nc.sync)
                xbar_T(nj + 2, 1, nc.sync)
                xbar_T(nj + 2, 2, nc.scalar)
                xbar_T(nj + 2, 3, nc.scalar)

            at_cur = at_tiles[nj]
            tt = ttp.tile([P, FT, NCHUNK], bf16, tag="tt")
            tts[nj] = tt
            pt = [
                ps_mm.tile([P, NCHUNK], f32, tag="psm", name=f"pt_{nj}_{fi}")
                for fi in range(FT)
            ]
            for fi in range(FT):
                for mi in range(NT):
                    nc.tensor.matmul(
                        pt[fi][:],
                        lhsT=x_b[:, mi, fi * P : (fi + 1) * P],
                        rhs=at_cur[:, mi, :, :],
                        start=(mi == 0),
                        stop=(mi == NT - 1),
                    )
                copyback(tt[:, fi], pt[fi][:])
                # mm2 of the previous chunk, one fi-group per mm1 fi-block:
                # inputs landed a full chunk ago, so the PE never waits.
                if nj > 0:
                    mm2_group(nj - 1, fi)

        for fi in range(FT):
            mm2_group(NJ - 1, fi)

    nc.compile()
    return nc


def get_nc():
    if "nc" not in _CACHE:
        _CACHE["nc"] = _build_nc()
    return _CACHE["nc"]


def kernel(**inputs) -> np.ndarray:
    from concourse.bass_utils import run_bass_kernel_spmd

    x = np.ascontiguousarray(np.asarray(inputs["x"], dtype=np.float32))
    a = np.ascontiguousarray(np.asarray(inputs["a"], dtype=np.float32))
    w = np.ascontiguousarray(np.asarray(inputs["kernel"], dtype=np.float32))
    assert x.shape == (B, N, F) and a.shape == (B, N, N) and w.shape == (F, D)

    nc = get_nc()
    in_maps = [{"a": a[b], "x": x[b], "kernel": w} for b in range(B)]
    res = run_bass_kernel_spmd(nc, in_maps, core_ids=list(range(B)))
    return np.stack([res.results[b]["out"] for b in range(B)], axis=0)
